# revision 3
# baseline (speedup 1.0000x reference)
"""GAU denoising transformer forward pass on 8 Trainium2 NeuronCores.

Strategy: data-parallel over batch (B=16 -> 2 images per core). Each core
runs an identical NEFF on its own pair of images with all weights
replicated. Per core the residual stream is kept transposed in SBUF
(hT: H on partitions x 512 tokens = 2 images x 256 patches) in fp32 for
the whole 24-layer stack; weights are streamed in bf16 and all GEMMs run
in bf16 with fp32 PSUM accumulation.

Layout choices per layer:
  - uvqk GEMM is computed weight-stationary producing transposed outputs
    (feature on partitions) for u / q / k. The q/k columns are duplicated
    with swapped halves ("qswap"/"kswap" columns) so RoPE becomes three
    partition-aligned vector ops (no cross-partition reads).
  - v is computed activation-stationary producing the natural layout
    (token on partitions), which is what attn@v needs as stationary.
  - softmax over the free axis; attn (128x256 per l-chunk) transposed via
    the PE transpose; attn@v gives oT (feature on partitions); gating with
    uT; out-projection accumulates back into hT.
  - rmsnorm reduction over H (the partition axis) is done with a
    ones-vector matmul over squared activations; gnorm / fnorm_w are
    folded into the weight matrices on the host.
"""

import sys

for _p in ("/opt/trn_rl_repo",):
    if _p not in sys.path:
        sys.path.append(_p)

import numpy as np
import ml_dtypes

BF = ml_dtypes.bfloat16

IMG = 128
P = 8
H = 768
E = 1536
KD = 128          # key size
L = 256           # patches per image
PD = 192          # patch dim
NL = 24
B = 16
NCORES = 8
TOK = 512         # tokens per core (2 images x 256)
HC = H // 128     # 6 h-chunks
EC = E // 128     # 12 e-chunks
WUV_W = E + 4 * 128 + E   # permuted wuv width: u | q | qswap | k | kswap | v
V0 = E + 4 * 128          # column offset of v block


def _build(nl=NL, repeat=1):
    """Build + compile the Bass module. Returns nc."""
    import concourse.tile as tile
    from concourse import bacc, mybir
    from concourse.masks import make_identity

    F32 = mybir.dt.float32
    BF16 = mybir.dt.bfloat16
    AF = mybir.ActivationFunctionType

    nc = bacc.Bacc("TRN2", target_bir_lowering=False, debug=False,
                   num_devices=NCORES)

    d_xpt = nc.dram_tensor("xpt", [128, 2, TOK], BF16, kind="ExternalInput")
    d_temb = nc.dram_tensor("temb", [128, HC, 2], F32, kind="ExternalInput")
    d_pw = nc.dram_tensor("pw", [128, 2, H], BF16, kind="ExternalInput")
    d_wuv = nc.dram_tensor("wuv", [nl, 128, HC, WUV_W], BF16,
                           kind="ExternalInput")
    d_wout = nc.dram_tensor("wout", [nl, 128, EC, H], BF16,
                            kind="ExternalInput")
    d_upw = nc.dram_tensor("upw", [128, HC, PD], BF16, kind="ExternalInput")
    d_cq = nc.dram_tensor("cq", [128, TOK], F32, kind="ExternalInput")
    d_sq = nc.dram_tensor("sq", [128, TOK], F32, kind="ExternalInput")
    d_ck = nc.dram_tensor("ck", [128, TOK], F32, kind="ExternalInput")
    d_sk = nc.dram_tensor("sk", [128, TOK], F32, kind="ExternalInput")
    d_out = nc.dram_tensor("outt", [PD, TOK], F32, kind="ExternalOutput")

    from contextlib import ExitStack

    with tile.TileContext(nc) as tc, ExitStack() as ctx:
        pers = ctx.enter_context(tc.tile_pool(name="pers", bufs=1))
        wuvp = ctx.enter_context(tc.tile_pool(name="wuvp", bufs=2))
        woutp = ctx.enter_context(tc.tile_pool(name="woutp", bufs=1))
        rtmp = ctx.enter_context(tc.tile_pool(name="rtmp", bufs=1))
        hsqp = ctx.enter_context(tc.tile_pool(name="hsqp", bufs=2))
        attnp = ctx.enter_context(tc.tile_pool(name="attnp", bufs=3))
        attntp = ctx.enter_context(tc.tile_pool(name="attntp", bufs=3))
        statp = ctx.enter_context(tc.tile_pool(name="statp", bufs=4))
        rmsp = ctx.enter_context(tc.tile_pool(name="rmsp", bufs=1))
        rbp = ctx.enter_context(tc.tile_pool(name="rbp", bufs=1))

        psum = ctx.enter_context(tc.tile_pool(name="psum", bufs=1, space="PSUM"))

        # ---- persistent state + constants ----
        hT = pers.tile([128, HC, TOK], F32)        # residual stream (transposed)
        xnT = pers.tile([128, HC, TOK], BF16)      # normed activations
        uT = pers.tile([128, EC, TOK], BF16)
        vn = pers.tile([128, 4, E], BF16)          # v natural; 4 token chunks
        ogT = pers.tile([128, EC, TOK], BF16)      # gated o (transposed)
        qp = pers.tile([128, TOK], BF16)           # roped q (scaled)
        kp = pers.tile([128, TOK], BF16)           # roped k
        cq = pers.tile([128, TOK], F32)
        sq = pers.tile([128, TOK], F32)
        ck = pers.tile([128, TOK], F32)
        sk = pers.tile([128, TOK], F32)
        temb = pers.tile([128, HC, 2], F32)
        xpt = pers.tile([128, 2, TOK], BF16)
        pw = pers.tile([128, 2, H], BF16)
        upw = pers.tile([128, HC, PD], BF16)
        ones = pers.tile([128, 1], BF16)
        ident = pers.tile([128, 128], BF16)

        nc.sync.dma_start(cq, d_cq.ap())
        nc.sync.dma_start(sq, d_sq.ap())
        nc.sync.dma_start(ck, d_ck.ap())
        nc.sync.dma_start(sk, d_sk.ap())
        nc.sync.dma_start(temb, d_temb.ap())
        nc.sync.dma_start(xpt, d_xpt.ap())
        nc.sync.dma_start(pw, d_pw.ap())
        nc.sync.dma_start(upw, d_upw.ap())
        nc.vector.memset(ones, 1.0)
        make_identity(nc, ident)

        # ---- patchify: hT = patch_W.T @ xp.T + temb ----
        for j in range(HC):
            ps = psum.tile([128, TOK], F32, tag=f"p{1 + j % 2}")
            for c in range(2):
                nc.tensor.matmul(ps, pw[:, c, j * 128:(j + 1) * 128],
                                 xpt[:, c, :], start=(c == 0), stop=(c == 1))
            for i in range(2):
                nc.vector.tensor_scalar_add(
                    hT[:, j, i * 256:(i + 1) * 256],
                    ps[:, i * 256:(i + 1) * 256],
                    temb[:, j, i:i + 1])

        def rms_recip_broadcast():
            """sum over H of hT^2 -> 1/(rms+eps) broadcast to 128 partitions."""
            ss = psum.tile([1, TOK], F32, tag="p0")
            for j in range(HC):
                hsq = hsqp.tile([128, TOK], BF16, tag="hsq")
                nc.scalar.square(hsq, hT[:, j, :])
                nc.tensor.matmul(ss, ones, hsq, start=(j == 0),
                                 stop=(j == HC - 1))
            rms = rmsp.tile([1, TOK], F32, tag="rms")
            nc.scalar.activation(rms, ss, AF.Sqrt, scale=1.0 / H)
            nc.vector.tensor_scalar_add(rms, rms, 1e-6)
            rinv = rmsp.tile([1, TOK], F32, tag="rinv")
            nc.vector.reciprocal(rinv, rms)
            rb = rbp.tile([128, TOK], F32, tag="rb")
            nc.gpsimd.partition_broadcast(rb, rinv)
            return rb

        for lrep in range(nl * repeat):
            li = lrep % nl
            wuv = wuvp.tile([128, HC, WUV_W], BF16, tag="wuv")
            nc.sync.dma_start(wuv, d_wuv.ap()[li])
            wout = woutp.tile([128, EC, H], BF16, tag="wout")
            nc.sync.dma_start(wout, d_wout.ap()[li])

            # ---- rmsnorm -> xnT (gnorm folded into weights on host) ----
            rb = rms_recip_broadcast()
            for j in range(HC):
                nc.vector.tensor_mul(xnT[:, j, :], hT[:, j, :], rb)

            # ---- q/qswap/k/kswap col-tiles (weight-stationary) ----
            qk_ps = []
            for t in range(4):
                ct0 = E + t * 128
                ps = psum.tile([128, TOK], F32, tag=f"p{1 + t}")
                for j in range(HC):
                    nc.tensor.matmul(ps, wuv[:, j, ct0:ct0 + 128],
                                     xnT[:, j, :], start=(j == 0),
                                     stop=(j == HC - 1))
                qk_ps.append(ps)

            # ---- rope (partition aligned):
            #   q' = Q*cos_q + Qswap*sins_q ; k' = K*cos_k + Kswap*sins_k
            m1 = rtmp.tile([128, TOK], F32, tag="m1")
            m2 = rtmp.tile([128, TOK], F32, tag="m2")
            nc.vector.tensor_mul(m1, qk_ps[0], cq)
            nc.vector.tensor_mul(m2, qk_ps[1], sq)
            nc.vector.tensor_add(qp, m1, m2)
            m3 = rtmp.tile([128, TOK], F32, tag="m1")
            m4 = rtmp.tile([128, TOK], F32, tag="m2")
            nc.vector.tensor_mul(m3, qk_ps[2], ck)
            nc.vector.tensor_mul(m4, qk_ps[3], sk)
            nc.vector.tensor_add(kp, m3, m4)

            # ---- v natural (activation-stationary) ----
            for tk in range(4):
                for ns in range(3):
                    ps = psum.tile([128, 512], F32, tag=f"p{5 + (tk * 3 + ns) % 2}")
                    for j in range(HC):
                        nc.tensor.matmul(
                            ps,
                            xnT[:, j, tk * 128:(tk + 1) * 128],
                            wuv[:, j, V0 + ns * 512:V0 + (ns + 1) * 512],
                            start=(j == 0), stop=(j == HC - 1))
                    nc.scalar.activation(vn[:, tk, ns * 512:(ns + 1) * 512],
                                         ps, AF.Silu)

            # ---- scores + softmax (per image, per l-chunk) ----
            attn_sb = {}
            for i in range(2):
                for c in range(2):
                    sc = psum.tile([128, 256], F32, tag=("p7", "p0")[(i * 2 + c) % 2])
                    nc.tensor.matmul(sc, qp[:, i * 256 + c * 128:
                                            i * 256 + (c + 1) * 128],
                                     kp[:, i * 256:(i + 1) * 256],
                                     start=True, stop=True)
                    nmax = statp.tile([128, 1], F32, tag="nmax")
                    nc.vector.reduce_max(nmax, sc,
                                         axis=mybir.AxisListType.X,
                                         negate=True)
                    at = attnp.tile([128, 256], BF16, tag="attn")
                    sume = statp.tile([128, 1], F32, tag="sume")
                    nc.scalar.activation(at, sc, AF.Exp, bias=nmax,
                                         scale=1.0, accum_out=sume)
                    rec = statp.tile([128, 1], F32, tag="rec")
                    nc.vector.reciprocal(rec, sume)
                    nc.vector.tensor_scalar_mul(at, at, rec)
                    attn_sb[(i, c)] = at

            # ---- u col-tiles (weight-stationary) ----
            for ct in range(EC):
                ps = psum.tile([128, TOK], F32, tag=f"p{1 + ct % 4}")
                for j in range(HC):
                    nc.tensor.matmul(ps, wuv[:, j, ct * 128:(ct + 1) * 128],
                                     xnT[:, j, :], start=(j == 0),
                                     stop=(j == HC - 1))
                nc.scalar.activation(uT[:, ct, :], ps, AF.Silu)

            # ---- transpose attn (PE) ----
            attnT = {}
            for i in range(2):
                for m in range(2):
                    aps = psum.tile([128, 256], BF16, tag=f"p{5 + (i * 2 + m) % 2}")
                    for c in range(2):
                        nc.tensor.transpose(
                            aps[:, c * 128:(c + 1) * 128],
                            attn_sb[(i, c)][:, m * 128:(m + 1) * 128],
                            ident)
                    asb = attntp.tile([128, 256], BF16, tag="ats")
                    nc.vector.tensor_copy(asb, aps)
                    attnT[(i, m)] = asb

            # ---- oT = (attn @ v).T ; gate with uT ----
            for i in range(2):
                for e in range(EC):
                    ops = psum.tile([128, 256], F32, tag=("p7", "p0")[(i * EC + e) % 2])
                    for m in range(2):
                        nc.tensor.matmul(ops,
                                         vn[:, i * 2 + m,
                                            e * 128:(e + 1) * 128],
                                         attnT[(i, m)],
                                         start=(m == 0), stop=(m == 1))
                    nc.vector.tensor_mul(ogT[:, e, i * 256:(i + 1) * 256],
                                         uT[:, e, i * 256:(i + 1) * 256],
                                         ops)

            # ---- out-projection + residual ----
            for hp in range(HC):
                dps = psum.tile([128, TOK], F32, tag=f"p{1 + hp % 4}")
                for e in range(EC):
                    nc.tensor.matmul(dps, wout[:, e, hp * 128:(hp + 1) * 128],
                                     ogT[:, e, :], start=(e == 0),
                                     stop=(e == EC - 1))
                nc.vector.tensor_add(hT[:, hp, :], hT[:, hp, :], dps)

        # ---- final norm + unpatch (fnorm_w folded into upw on host) ----
        rb = rms_recip_broadcast()
        for j in range(HC):
            nc.vector.tensor_mul(xnT[:, j, :], hT[:, j, :], rb)
        for mchunk, msz in ((0, 128), (1, 64)):
            ps = psum.tile([128, TOK], F32, tag=f"p{5 + mchunk}")
            for j in range(HC):
                nc.tensor.matmul(ps[:msz, :],
                                 upw[:, j, mchunk * 128:mchunk * 128 + msz],
                                 xnT[:, j, :], start=(j == 0),
                                 stop=(j == HC - 1))
            osb = rtmp.tile([128, TOK], F32, tag="m1")
            nc.vector.tensor_copy(osb[:msz, :], ps[:msz, :])
            nc.sync.dma_start(d_out.ap()[mchunk * 128:mchunk * 128 + msz, :],
                              osb[:msz, :])

    nc.compile()
    return nc


_BUILD_CACHE = {}


def _get_nc(nl=NL, repeat=1):
    key = (nl, repeat)
    if key not in _BUILD_CACHE:
        _BUILD_CACHE[key] = _build(nl, repeat)
    return _BUILD_CACHE[key]


def _rope_tables():
    pos = np.arange(L)

    def sinemb(p, dim=64, base=1000.0):
        half = dim // 2
        freqs = np.exp(np.arange(half, dtype=np.float32)
                       * np.float32(-np.log(base) / (half - 1)))
        ang = p[:, None].astype(np.float32) * freqs[None, :]
        return np.concatenate([np.sin(ang), np.cos(ang)], axis=-1)

    w = IMG // P
    pe = np.concatenate([sinemb(pos // w), sinemb(pos % w)],
                        axis=-1).astype(np.float32)      # (256, 128)
    sinv = pe[:, :64].T                                  # (64, 256)
    cosv = pe[:, 64:].T
    COS = np.concatenate([cosv, cosv], axis=0)           # (128, 256)
    SINS = np.concatenate([-sinv, sinv], axis=0)
    COS2 = np.tile(COS, (1, 2))                          # (128, 512)
    SINS2 = np.tile(SINS, (1, 2))
    scale = np.float32(KD ** -0.5)
    return (np.ascontiguousarray(COS2 * scale), np.ascontiguousarray(SINS2 * scale),
            np.ascontiguousarray(COS2), np.ascontiguousarray(SINS2))


def _prep_weights(patch_W, t_emb, Wuv, Wout, gnorm, fnorm_w, unpatch_W, nl=NL):
    Wg = Wuv[:nl] * gnorm[:nl, :, None]                  # fold gnorm
    u = Wg[:, :, :E]
    q = Wg[:, :, 2 * E:2 * E + KD]
    k = Wg[:, :, 2 * E + KD:]
    v = Wg[:, :, E:2 * E]
    qs = np.concatenate([q[:, :, 64:], q[:, :, :64]], axis=2)
    ks = np.concatenate([k[:, :, 64:], k[:, :, :64]], axis=2)
    wuvp = np.concatenate([u, q, qs, k, ks, v], axis=2)  # (nl, 768, 3584)
    wuv_h = np.ascontiguousarray(
        wuvp.reshape(nl, HC, 128, WUV_W).transpose(0, 2, 1, 3)).astype(BF)
    wout_h = np.ascontiguousarray(
        Wout[:nl].reshape(nl, EC, 128, H).transpose(0, 2, 1, 3)).astype(BF)
    pw_pad = np.zeros((256, H), np.float32)
    pw_pad[:PD] = patch_W
    pw_h = np.ascontiguousarray(
        pw_pad.reshape(2, 128, H).transpose(1, 0, 2)).astype(BF)
    upw = fnorm_w[:, None] * unpatch_W                   # fold fnorm
    upw_h = np.ascontiguousarray(
        upw.reshape(HC, 128, PD).transpose(1, 0, 2)).astype(BF)
    return wuv_h, wout_h, pw_h, upw_h


def _patchify(xc):
    """(2,3,128,128) -> (512, 192) token-major patches."""
    g = IMG // P
    xp = xc.reshape(2, 3, g, P, g, P).transpose(0, 2, 4, 3, 5, 1)
    return np.ascontiguousarray(xp.reshape(2 * L, PD))


def _unpatchify(oT):
    """(192, 512) -> (2, 3, 128, 128)."""
    g = IMG // P
    out = np.empty((2, 3, IMG, IMG), np.float32)
    for i in range(2):
        h = oT[:, i * L:(i + 1) * L].T                   # (256, 192)
        out[i] = (h.reshape(g, g, P, P, 3)
                  .transpose(4, 0, 2, 1, 3).reshape(3, IMG, IMG))
    return out


def make_in_maps(x, t_idx, patch_W, t_emb, Wuv, Wout, gnorm, fnorm_w,
                 unpatch_W, nl=NL):
    x = np.asarray(x, np.float32)
    t_idx = np.asarray(t_idx).astype(np.int64)
    patch_W = np.asarray(patch_W, np.float32)
    t_emb = np.asarray(t_emb, np.float32)
    Wuv = np.asarray(Wuv, np.float32)
    Wout = np.asarray(Wout, np.float32)
    gnorm = np.asarray(gnorm, np.float32)
    fnorm_w = np.asarray(fnorm_w, np.float32)
    unpatch_W = np.asarray(unpatch_W, np.float32)

    wuv_h, wout_h, pw_h, upw_h = _prep_weights(
        patch_W, t_emb, Wuv, Wout, gnorm, fnorm_w, unpatch_W, nl)
    cqt, sqt, ckt, skt = _rope_tables()

    in_maps = []
    for c in range(NCORES):
        xc = x[2 * c:2 * c + 2]
        xp = _patchify(xc)                               # (512, 192)
        xpad = np.zeros((TOK, 256), np.float32)
        xpad[:, :PD] = xp
        xpt = np.ascontiguousarray(
            xpad.T.reshape(2, 128, TOK).transpose(1, 0, 2)).astype(BF)
        te = t_emb[t_idx[2 * c:2 * c + 2, 0]]            # (2, 768)
        tembT = np.ascontiguousarray(
            te.T.reshape(HC, 128, 2).transpose(1, 0, 2)).astype(np.float32)
        in_maps.append({
            "xpt": xpt, "temb": tembT, "pw": pw_h, "wuv": wuv_h,
            "wout": wout_h, "upw": upw_h, "cq": cqt, "sq": sqt,
            "ck": ckt, "sk": skt,
        })
    return in_maps


def kernel(**inputs):
    from concourse.bass_utils import run_bass_kernel_spmd

    nc = _get_nc()
    in_maps = make_in_maps(**inputs)
    res = run_bass_kernel_spmd(nc, in_maps, core_ids=list(range(NCORES)))
    out = np.empty((B, 3, IMG, IMG), np.float32)
    for c in range(NCORES):
        out[2 * c:2 * c + 2] = _unpatchify(res.results[c]["outt"])
    return out
